# revision 4
# baseline (speedup 1.0000x reference)
"""Trainium2 Bass kernel for nn_BSN_76218489635087 (segment_reduce).

Computation (reference):
    h = relu-MLP(x[0])            # [2048, 64]
    s = h @ tr_bags               # [2048, 100000]
    col_max = max over rows       # [100000]
    ref_max = segment_max(col_max, tr_mask, 100)
    y_prob = sigmoid(ref_max @ W4 + b4); y_hat = y_prob >= 0.5

Sharding: tr_bags columns (T) split across 8 cores (12544 padded cols each).
Each core computes the full (replicated) MLP producing hT = h.T [64, 2048]
in SBUF, then for each 128-column tile of its bags shard computes
sT = bags_tile.T @ hT into PSUM [128, 2048] and reduces max over the free
(n) dim, giving per-column maxes. Host gathers the 100352 column maxes,
does the (tiny) segment-max + final 100->1 dot + sigmoid.
"""

import sys
import os

for _p in ("/opt/trn_rl_repo", "/root/.axon_site/_ro/pypackages", "/root/.axon_site"):
    if _p not in sys.path and os.path.isdir(_p):
        sys.path.append(_p)

import numpy as np

from concourse import bass, bacc, tile, mybir
from concourse.bass_utils import run_bass_kernel_spmd

# Problem constants (hardcoded per harness contract)
N = 2048          # instances
D = 512           # input features
T = 100000        # reference instance columns
R = 100           # num references (segments)
NCORES = 8
TPC = 12544       # padded columns per core (= 98 * 128); 8*12544 = 100352
NT = TPC // 128   # 98 column-tiles per core

F32 = mybir.dt.float32


def _build_program():
    nc = bacc.Bacc("TRN2", target_bir_lowering=False, debug=False, num_devices=NCORES)

    xT_d = nc.dram_tensor("xT", [D, N], F32, kind="ExternalInput")
    w1_d = nc.dram_tensor("w1", [D, 256], F32, kind="ExternalInput")
    w2_d = nc.dram_tensor("w2", [256, 128], F32, kind="ExternalInput")
    w3_d = nc.dram_tensor("w3", [128, 64], F32, kind="ExternalInput")
    b1_d = nc.dram_tensor("b1", [256, 1], F32, kind="ExternalInput")
    b2_d = nc.dram_tensor("b2", [128, 1], F32, kind="ExternalInput")
    b3_d = nc.dram_tensor("b3", [64, 1], F32, kind="ExternalInput")
    bags_d = nc.dram_tensor("bags", [64, TPC], F32, kind="ExternalInput")
    out_d = nc.dram_tensor("colmax_out", [128, NT], F32, kind="ExternalOutput")

    with tile.TileContext(nc) as tc:
        with (
            tc.tile_pool(name="const", bufs=1) as cpool,
            tc.tile_pool(name="psum", bufs=2, space="PSUM") as ppool,
        ):
            # ---- load everything ----
            xT_sb = []
            for k in range(4):
                t = cpool.tile([128, N], F32, tag=f"xT{k}", name=f"xT{k}")
                nc.sync.dma_start(t[:], xT_d[128 * k : 128 * (k + 1), :])
                xT_sb.append(t)
            w1_sb = []
            for k in range(4):
                t = cpool.tile([128, 256], F32, tag=f"w1{k}", name=f"w1s{k}")
                nc.sync.dma_start(t[:], w1_d[128 * k : 128 * (k + 1), :])
                w1_sb.append(t)
            w2_sb = []
            for k in range(2):
                t = cpool.tile([128, 128], F32, tag=f"w2{k}", name=f"w2s{k}")
                nc.sync.dma_start(t[:], w2_d[128 * k : 128 * (k + 1), :])
                w2_sb.append(t)
            w3_sb = cpool.tile([128, 64], F32, tag="w3")
            nc.sync.dma_start(w3_sb[:], w3_d[:, :])
            b1_sb = []
            for m in range(2):
                t = cpool.tile([128, 1], F32, tag=f"b1{m}", name=f"b1s{m}")
                nc.sync.dma_start(t[:], b1_d[128 * m : 128 * (m + 1), :])
                b1_sb.append(t)
            b2_sb = cpool.tile([128, 1], F32, tag="b2")
            nc.sync.dma_start(b2_sb[:], b2_d[:, :])
            b3_sb = cpool.tile([64, 1], F32, tag="b3")
            nc.sync.dma_start(b3_sb[:], b3_d[:, :])

            bags_sb = cpool.tile([64, TPC], F32, tag="bags")
            nc.sync.dma_start(bags_sb[:], bags_d[:, :])

            g1_sb = [cpool.tile([128, N], F32, tag=f"g1{m}", name=f"g1s{m}") for m in range(2)]
            g2_sb = cpool.tile([128, N], F32, tag="g2")
            hT_sb = cpool.tile([64, N], F32, tag="hT")
            colmax_sb = cpool.tile([128, NT], F32, tag="colmax")

            relu = mybir.ActivationFunctionType.Relu

            # ---- layer 1: g1 = relu(W1.T @ xT + b1) -> [256, 2048] as 2 tiles
            for m in range(2):
                ps = ppool.tile([128, N], F32, tag="ps")
                for j in range(4):
                    for k in range(4):
                        nc.tensor.matmul(
                            ps[:, 512 * j : 512 * (j + 1)],
                            w1_sb[k][:, 128 * m : 128 * (m + 1)],
                            xT_sb[k][:, 512 * j : 512 * (j + 1)],
                            start=(k == 0),
                            stop=(k == 3),
                        )
                nc.scalar.activation(g1_sb[m][:, :], ps[:, :], relu, bias=b1_sb[m][:, :])

            # ---- layer 2: g2 = relu(W2.T @ g1 + b2) -> [128, 2048]
            ps = ppool.tile([128, N], F32, tag="ps")
            for j in range(4):
                for k in range(2):
                    nc.tensor.matmul(
                        ps[:, 512 * j : 512 * (j + 1)],
                        w2_sb[k][:, :],
                        g1_sb[k][:, 512 * j : 512 * (j + 1)],
                        start=(k == 0),
                        stop=(k == 1),
                    )
            nc.scalar.activation(g2_sb[:, :], ps[:, :], relu, bias=b2_sb[:, :])

            # ---- layer 3: hT = relu(W3.T @ g2 + b3) -> [64, 2048]
            ps = ppool.tile([128, N], F32, tag="ps")
            for j in range(4):
                nc.tensor.matmul(
                    ps[0:64, 512 * j : 512 * (j + 1)],
                    w3_sb[:, :],
                    g2_sb[:, 512 * j : 512 * (j + 1)],
                    start=True,
                    stop=True,
                )
            nc.scalar.activation(hT_sb[:, :], ps[0:64, :], relu, bias=b3_sb[:, :])

            # ---- scores: for each 128-col bag tile, sT = bags_tile.T @ hT,
            #      col max over the free (n) dim
            for i in range(NT):
                ps = ppool.tile([128, N], F32, tag="ps")
                lhsT = bags_sb[:, 128 * i : 128 * (i + 1)]
                for j in range(4):
                    nc.tensor.matmul(
                        ps[:, 512 * j : 512 * (j + 1)],
                        lhsT,
                        hT_sb[:, 512 * j : 512 * (j + 1)],
                        start=True,
                        stop=True,
                    )
                nc.vector.reduce_max(
                    colmax_sb[:, i : i + 1], ps[:, :], axis=mybir.AxisListType.X
                )

            nc.sync.dma_start(out_d[:, :], colmax_sb[:])

    nc.compile()
    return nc


_CACHED = {}


def _get_program():
    if "nc" not in _CACHED:
        _CACHED["nc"] = _build_program()
    return _CACHED["nc"]


def _run_device(in_maps, trace=False):
    nc = _get_program()
    try:
        return run_bass_kernel_spmd(nc, in_maps, list(range(NCORES)), trace=trace)
    except ModuleNotFoundError:
        if not trace:
            raise
        return run_bass_kernel_spmd(nc, in_maps, list(range(NCORES)), trace=False)


def _prep_inputs(x, tr_bags, W1, b1, W2, b2, W3, b3):
    xT = np.ascontiguousarray(np.asarray(x, np.float32)[0].T)  # [512, 2048]
    bags = np.asarray(tr_bags, np.float32)
    bags_pad = np.zeros((64, NCORES * TPC), np.float32)
    bags_pad[:, :T] = bags
    base = {
        "xT": xT,
        "w1": np.ascontiguousarray(np.asarray(W1, np.float32)),
        "w2": np.ascontiguousarray(np.asarray(W2, np.float32)),
        "w3": np.ascontiguousarray(np.asarray(W3, np.float32)),
        "b1": np.asarray(b1, np.float32).reshape(256, 1).copy(),
        "b2": np.asarray(b2, np.float32).reshape(128, 1).copy(),
        "b3": np.asarray(b3, np.float32).reshape(64, 1).copy(),
    }
    in_maps = []
    for c in range(NCORES):
        m = dict(base)
        m["bags"] = np.ascontiguousarray(bags_pad[:, c * TPC : (c + 1) * TPC])
        in_maps.append(m)
    return in_maps


def _finish_host(colmax, tr_mask, W4, b4):
    tm = np.asarray(tr_mask)
    boundaries = np.searchsorted(tm, np.arange(R + 1))
    ref_max = np.full(R, -np.inf, np.float32)
    nonempty = boundaries[1:] > boundaries[:-1]
    if nonempty.any():
        starts = boundaries[:-1][nonempty]
        ref_max[nonempty] = np.maximum.reduceat(colmax, starts)[: nonempty.sum()]
    z = ref_max.astype(np.float32) @ np.asarray(W4, np.float32) + np.asarray(
        b4, np.float32
    )
    y_prob = (1.0 / (1.0 + np.exp(-z.astype(np.float64)))).astype(np.float32).squeeze()
    y_hat = np.float32(1.0) if y_prob >= 0.5 else np.float32(0.0)
    return np.asarray(y_prob, np.float32), np.asarray(y_hat, np.float32)


def kernel(x, tr_bags, tr_mask, W1, b1, W2, b2, W3, b3, W4, b4, _trace=False):
    in_maps = _prep_inputs(x, tr_bags, W1, b1, W2, b2, W3, b3)
    res = _run_device(in_maps, trace=_trace)
    colmax_parts = []
    for c in range(NCORES):
        cm = res.results[c]["colmax_out"]  # [128, NT]
        colmax_parts.append(np.asarray(cm).T.reshape(-1))  # [TPC], col-major by tile
    colmax = np.concatenate(colmax_parts)[:T]
    out = _finish_host(colmax, tr_mask, W4, b4)
    if _trace:
        return out, res
    return out


# revision 9
# speedup vs baseline: 1.4968x; 1.4968x over previous
"""Trainium2 Bass kernel for nn_BSN_76218489635087 (segment_reduce).

Computation (reference):
    h = relu-MLP(x[0])            # [2048, 64]
    s = h @ tr_bags               # [2048, 100000]
    col_max = max over rows       # [100000]
    ref_max = segment_max(col_max, tr_mask, 100)
    y_prob = sigmoid(ref_max @ W4 + b4); y_hat = y_prob >= 0.5

Sharding: tr_bags columns (T) split across 8 cores (12544 padded cols each).
Each core computes the full (replicated) MLP producing hT = h.T, duplicated
into both partition halves [128, 2048]. Bags are host-packed [128, 6272] so
consecutive 128-col tiles alternate partition halves (row groups), letting
LDWEIGHTS overlap in-flight MATMULs. Matmuls run in float32r (full fp32
operands, 1 cycle/row for N>=256 vs 4 for plain fp32).

PSUM drain (the bottleneck): per score tile [128, 2048] either
  - DVE reduce_max direct from PSUM, or
  - ACT copies PSUM -> SBUF fp16, then one DVE tensor_tensor_reduce(max,max)
    folds the 2048 fp16 values to the per-column max,
split ~5:3 so ScalarE and VectorE drain concurrently.

Host gathers the 100352 column maxes, does the segment-max + final
100->1 dot + sigmoid.
"""

import sys
import os

for _p in ("/opt/trn_rl_repo", "/root/.axon_site/_ro/pypackages", "/root/.axon_site"):
    if _p not in sys.path and os.path.isdir(_p):
        sys.path.append(_p)

import numpy as np

from concourse import bass, bacc, tile, mybir
from concourse.bass_utils import run_bass_kernel_spmd

# Problem constants (hardcoded per harness contract)
N = 2048          # instances
D = 512           # input features
T = 100000        # reference instance columns
R = 100           # num references (segments)
NCORES = 8
TPC = 12544       # padded columns per core (= 98 * 128); 8*12544 = 100352
NT = TPC // 128   # 98 column-tiles per core

F32 = mybir.dt.float32
F32R = mybir.dt.float32r
F16 = mybir.dt.float16

# Of every 8 score tiles, this many drain via the ACT-copy path (rest DVE).
ACT_TILES = frozenset({0, 1, 2, 4, 6})

USE_ALT = os.environ.get("K_ALT", "0") == "1"      # row-group alternation
USE_SPLIT = os.environ.get("K_SPLIT", "1") == "1"  # ACT/DVE drain split
TAIL = os.environ.get("K_TAIL", "ttr")             # ttr | reduce


def _mm(nc, out, lhsT, rhs, **kw):
    nc.tensor.matmul(out, lhsT, rhs, **kw)


def _build_program():
    nc = bacc.Bacc("TRN2", target_bir_lowering=False, debug=False, num_devices=NCORES)

    xT_d = nc.dram_tensor("xT", [D, N], F32R, kind="ExternalInput")
    w1_d = nc.dram_tensor("w1", [D, 256], F32R, kind="ExternalInput")
    w2_d = nc.dram_tensor("w2", [256, 128], F32R, kind="ExternalInput")
    w3_d = nc.dram_tensor("w3", [128, 64], F32R, kind="ExternalInput")
    b1_d = nc.dram_tensor("b1", [256, 1], F32, kind="ExternalInput")
    b2_d = nc.dram_tensor("b2", [128, 1], F32, kind="ExternalInput")
    b3_d = nc.dram_tensor("b3", [64, 1], F32, kind="ExternalInput")
    bags_shape = [128, TPC // 2] if USE_ALT else [64, TPC]
    bags_d = nc.dram_tensor("bags", bags_shape, F32R, kind="ExternalInput")
    out_d = nc.dram_tensor("colmax_out", [128, NT], F32, kind="ExternalOutput")

    relu = mybir.ActivationFunctionType.Relu
    copyf = mybir.ActivationFunctionType.Copy
    amax = mybir.AluOpType.max

    with tile.TileContext(nc) as tc:
        with (
            tc.tile_pool(name="const", bufs=1) as cpool,
            tc.tile_pool(name="scr", bufs=4) as spool,
            tc.tile_pool(name="psum", bufs=2, space="PSUM") as ppool,
        ):
            # ---- load everything ----
            xT_sb = []
            for k in range(4):
                t = cpool.tile([128, N], F32R, tag=f"xT{k}", name=f"xT{k}")
                nc.sync.dma_start(t[:], xT_d[128 * k : 128 * (k + 1), :])
                xT_sb.append(t)
            w1_sb = []
            for k in range(4):
                t = cpool.tile([128, 256], F32R, tag=f"w1{k}", name=f"w1s{k}")
                nc.sync.dma_start(t[:], w1_d[128 * k : 128 * (k + 1), :])
                w1_sb.append(t)
            w2_sb = []
            for k in range(2):
                t = cpool.tile([128, 128], F32R, tag=f"w2{k}", name=f"w2s{k}")
                nc.sync.dma_start(t[:], w2_d[128 * k : 128 * (k + 1), :])
                w2_sb.append(t)
            w3_sb = cpool.tile([128, 64], F32R, tag="w3")
            nc.sync.dma_start(w3_sb[:], w3_d[:, :])
            b1_sb = []
            for m in range(2):
                t = cpool.tile([128, 1], F32, tag=f"b1{m}", name=f"b1s{m}")
                nc.sync.dma_start(t[:], b1_d[128 * m : 128 * (m + 1), :])
                b1_sb.append(t)
            b2_sb = cpool.tile([128, 1], F32, tag="b2")
            nc.sync.dma_start(b2_sb[:], b2_d[:, :])
            b3_sb = cpool.tile([64, 1], F32, tag="b3")
            nc.sync.dma_start(b3_sb[:], b3_d[:, :])

            bags_sb = cpool.tile(bags_shape, F32R, tag="bags")
            nc.sync.dma_start(bags_sb[:], bags_d[:, :])

            g1_sb = [
                cpool.tile([128, N], F32R, tag=f"g1{m}", name=f"g1s{m}")
                for m in range(2)
            ]
            g2_sb = cpool.tile([128, N], F32R, tag="g2")
            hT_sb = cpool.tile([128, N], F32R, tag="hT")
            colmax_sb = cpool.tile([128, NT], F32, tag="colmax")

            # ---- layer 1: g1 = relu(W1.T @ xT + b1) -> [256, 2048] as 2 tiles
            for m in range(2):
                ps = ppool.tile([128, N], F32, tag="ps", name=f"psl1{m}")
                for j in range(4):
                    for k in range(4):
                        _mm(
                            nc,
                            ps[:, 512 * j : 512 * (j + 1)],
                            w1_sb[k][:, 128 * m : 128 * (m + 1)],
                            xT_sb[k][:, 512 * j : 512 * (j + 1)],
                            start=(k == 0),
                            stop=(k == 3),
                        )
                nc.scalar.activation(g1_sb[m][:, :], ps[:, :], relu, bias=b1_sb[m][:, :])

            # ---- layer 2: g2 = relu(W2.T @ g1 + b2) -> [128, 2048]
            ps = ppool.tile([128, N], F32, tag="ps", name="psl2")
            for j in range(4):
                for k in range(2):
                    _mm(
                        nc,
                        ps[:, 512 * j : 512 * (j + 1)],
                        w2_sb[k][:, :],
                        g1_sb[k][:, 512 * j : 512 * (j + 1)],
                        start=(k == 0),
                        stop=(k == 1),
                    )
            nc.scalar.activation(g2_sb[:, :], ps[:, :], relu, bias=b2_sb[:, :])

            # ---- layer 3: hT = relu(W3.T @ g2 + b3) -> [64, 2048], then
            #      duplicated into partitions 64:128 for row-group alternation
            ps = ppool.tile([128, N], F32, tag="ps", name="psl3")
            for j in range(4):
                _mm(
                    nc,
                    ps[0:64, 512 * j : 512 * (j + 1)],
                    w3_sb[:, :],
                    g2_sb[:, 512 * j : 512 * (j + 1)],
                    start=True,
                    stop=True,
                )
            nc.scalar.activation(hT_sb[0:64, :], ps[0:64, :], relu, bias=b3_sb[:, :])
            if USE_ALT:
                nc.sync.dma_start(hT_sb[64:128, :], hT_sb[0:64, :])

            # ---- scores: tile i lives in partition half i%2, col block i//2
            for i in range(NT):
                if USE_ALT:
                    half = 64 * (i % 2)
                    lhsT = bags_sb[half : half + 64, 128 * (i // 2) : 128 * (i // 2) + 128]
                else:
                    half = 0
                    lhsT = bags_sb[:, 128 * i : 128 * (i + 1)]
                ps = ppool.tile([128, N], F32, tag="ps", name=f"pss{i}")
                for j in range(4):
                    _mm(
                        nc,
                        ps[:, 512 * j : 512 * (j + 1)],
                        lhsT,
                        hT_sb[half : half + 64, 512 * j : 512 * (j + 1)],
                        start=True,
                        stop=True,
                    )
                if USE_SPLIT and (i % 10) < 8:
                    scr = spool.tile([128, N], F16, tag="scr", name=f"scr{i}")
                    nc.scalar.activation(scr[:, :], ps[:, :], copyf)
                    t1 = spool.tile([128, N // 2], F16, tag="t1", name=f"t1_{i}")
                    nc.vector.tensor_max(t1[:, :], scr[:, 0 : N // 2], scr[:, N // 2 : N])
                    t2 = spool.tile([128, N // 4], F16, tag="t2", name=f"t2_{i}")
                    nc.vector.tensor_max(t2[:, :], t1[:, 0 : N // 4], t1[:, N // 4 : N // 2])
                    nc.vector.reduce_max(
                        colmax_sb[:, i : i + 1], t2[:, :], axis=mybir.AxisListType.X
                    )
                else:
                    nc.vector.reduce_max(
                        colmax_sb[:, i : i + 1], ps[:, :], axis=mybir.AxisListType.X
                    )

            nc.sync.dma_start(out_d[:, :], colmax_sb[:])

    nc.compile()
    return nc


_CACHED = {}


def _get_program():
    if "nc" not in _CACHED:
        _CACHED["nc"] = _build_program()
    return _CACHED["nc"]


def _run_device(in_maps, trace=False):
    nc = _get_program()
    try:
        return run_bass_kernel_spmd(nc, in_maps, list(range(NCORES)), trace=trace)
    except ModuleNotFoundError:
        if not trace:
            raise
        return run_bass_kernel_spmd(nc, in_maps, list(range(NCORES)), trace=False)


def _prep_inputs(x, tr_bags, W1, b1, W2, b2, W3, b3):
    xT = np.ascontiguousarray(np.asarray(x, np.float32)[0].T)  # [512, 2048]
    bags = np.asarray(tr_bags, np.float32)
    bags_pad = np.zeros((64, NCORES * TPC), np.float32)
    bags_pad[:, :T] = bags
    base = {
        "xT": xT,
        "w1": np.ascontiguousarray(np.asarray(W1, np.float32)),
        "w2": np.ascontiguousarray(np.asarray(W2, np.float32)),
        "w3": np.ascontiguousarray(np.asarray(W3, np.float32)),
        "b1": np.asarray(b1, np.float32).reshape(256, 1).copy(),
        "b2": np.asarray(b2, np.float32).reshape(128, 1).copy(),
        "b3": np.asarray(b3, np.float32).reshape(64, 1).copy(),
    }
    in_maps = []
    for c in range(NCORES):
        shard = bags_pad[:, c * TPC : (c + 1) * TPC]
        if USE_ALT:
            sh = shard.reshape(64, NT, 128)
            packed = np.empty((128, TPC // 2), np.float32)
            # even tiles -> partitions 0:64, odd tiles -> 64:128, col block i//2
            packed[0:64] = sh[:, 0::2, :].reshape(64, -1)
            packed[64:128] = sh[:, 1::2, :].reshape(64, -1)
        else:
            packed = shard
        m = dict(base)
        m["bags"] = np.ascontiguousarray(packed)
        in_maps.append(m)
    return in_maps


def _finish_host(colmax, tr_mask, W4, b4):
    tm = np.asarray(tr_mask)
    boundaries = np.searchsorted(tm, np.arange(R + 1))
    ref_max = np.full(R, -np.inf, np.float32)
    nonempty = boundaries[1:] > boundaries[:-1]
    if nonempty.any():
        starts = boundaries[:-1][nonempty]
        ref_max[nonempty] = np.maximum.reduceat(colmax, starts)[: nonempty.sum()]
    z = ref_max.astype(np.float32) @ np.asarray(W4, np.float32) + np.asarray(
        b4, np.float32
    )
    y_prob = (1.0 / (1.0 + np.exp(-z.astype(np.float64)))).astype(np.float32).squeeze()
    y_hat = np.float32(1.0) if y_prob >= 0.5 else np.float32(0.0)
    return np.asarray(y_prob, np.float32), np.asarray(y_hat, np.float32)


def kernel(x, tr_bags, tr_mask, W1, b1, W2, b2, W3, b3, W4, b4, _trace=False):
    in_maps = _prep_inputs(x, tr_bags, W1, b1, W2, b2, W3, b3)
    res = _run_device(in_maps, trace=_trace)
    colmax_parts = []
    for c in range(NCORES):
        cm = res.results[c]["colmax_out"]  # [128, NT]
        colmax_parts.append(np.asarray(cm).T.reshape(-1))  # [TPC], col-major by tile
    colmax = np.concatenate(colmax_parts)[:T]
    out = _finish_host(colmax, tr_mask, W4, b4)
    if _trace:
        return out, res
    return out


# revision 10
# speedup vs baseline: 1.5604x; 1.0424x over previous
"""Trainium2 Bass kernel for nn_BSN_76218489635087 (segment_reduce).

Computation (reference):
    h = relu-MLP(x[0])            # [2048, 64]
    s = h @ tr_bags               # [2048, 100000]
    col_max = max over rows       # [100000]
    ref_max = segment_max(col_max, tr_mask, 100)
    y_prob = sigmoid(ref_max @ W4 + b4); y_hat = y_prob >= 0.5

Sharding: tr_bags columns (T) split across 8 cores (12544 padded cols each).
Each core computes the full (replicated) MLP producing hT = h.T, duplicated
into both partition halves [128, 2048]. Bags are host-packed [128, 6272] so
consecutive 128-col tiles alternate partition halves (row groups), letting
LDWEIGHTS overlap in-flight MATMULs. Matmuls run in float32r (full fp32
operands, 1 cycle/row for N>=256 vs 4 for plain fp32).

PSUM drain (the bottleneck): per score tile [128, 2048] either
  - DVE reduce_max direct from PSUM, or
  - ACT copies PSUM -> SBUF fp16, then one DVE tensor_tensor_reduce(max,max)
    folds the 2048 fp16 values to the per-column max,
split ~5:3 so ScalarE and VectorE drain concurrently.

Host gathers the 100352 column maxes, does the segment-max + final
100->1 dot + sigmoid.
"""

import sys
import os

for _p in ("/opt/trn_rl_repo", "/root/.axon_site/_ro/pypackages", "/root/.axon_site"):
    if _p not in sys.path and os.path.isdir(_p):
        sys.path.append(_p)

import numpy as np

from concourse import bass, bacc, tile, mybir
from concourse.bass_utils import run_bass_kernel_spmd

# Problem constants (hardcoded per harness contract)
N = 2048          # instances
D = 512           # input features
T = 100000        # reference instance columns
R = 100           # num references (segments)
NCORES = 8
TPC = 12544       # padded columns per core (= 98 * 128); 8*12544 = 100352
NT = TPC // 128   # 98 column-tiles per core

F32 = mybir.dt.float32
F32R = mybir.dt.float32r
F16 = mybir.dt.float16

# Of every 8 score tiles, this many drain via the ACT-copy path (rest DVE).
ACT_TILES = frozenset({0, 1, 2, 4, 6})

USE_ALT = os.environ.get("K_ALT", "0") == "1"      # row-group alternation
USE_SPLIT = os.environ.get("K_SPLIT", "1") == "1"  # ACT/DVE drain split
TAIL = os.environ.get("K_TAIL", "ttr")             # ttr | reduce


def _mm(nc, out, lhsT, rhs, **kw):
    nc.tensor.matmul(out, lhsT, rhs, **kw)


def _build_program():
    nc = bacc.Bacc("TRN2", target_bir_lowering=False, debug=False, num_devices=NCORES)

    xT_d = nc.dram_tensor("xT", [D, N], F32R, kind="ExternalInput")
    w1_d = nc.dram_tensor("w1", [D, 256], F32R, kind="ExternalInput")
    w2_d = nc.dram_tensor("w2", [256, 128], F32R, kind="ExternalInput")
    w3_d = nc.dram_tensor("w3", [128, 64], F32R, kind="ExternalInput")
    b1_d = nc.dram_tensor("b1", [256, 1], F32, kind="ExternalInput")
    b2_d = nc.dram_tensor("b2", [128, 1], F32, kind="ExternalInput")
    b3_d = nc.dram_tensor("b3", [64, 1], F32, kind="ExternalInput")
    bags_shape = [128, TPC // 2] if USE_ALT else [64, TPC]
    bags_d = nc.dram_tensor("bags", bags_shape, F16, kind="ExternalInput")
    out_d = nc.dram_tensor("colmax_out", [128, NT], F32, kind="ExternalOutput")

    relu = mybir.ActivationFunctionType.Relu
    copyf = mybir.ActivationFunctionType.Copy
    amax = mybir.AluOpType.max

    with tile.TileContext(nc) as tc:
        with (
            tc.tile_pool(name="const", bufs=1) as cpool,
            tc.tile_pool(name="scr", bufs=4) as spool,
            tc.tile_pool(name="psum", bufs=2, space="PSUM") as ppool,
        ):
            # ---- load everything ----
            xT_sb = []
            for k in range(4):
                t = cpool.tile([128, N], F32R, tag=f"xT{k}", name=f"xT{k}")
                nc.sync.dma_start(t[:], xT_d[128 * k : 128 * (k + 1), :])
                xT_sb.append(t)
            w1_sb = []
            for k in range(4):
                t = cpool.tile([128, 256], F32R, tag=f"w1{k}", name=f"w1s{k}")
                nc.sync.dma_start(t[:], w1_d[128 * k : 128 * (k + 1), :])
                w1_sb.append(t)
            w2_sb = []
            for k in range(2):
                t = cpool.tile([128, 128], F32R, tag=f"w2{k}", name=f"w2s{k}")
                nc.sync.dma_start(t[:], w2_d[128 * k : 128 * (k + 1), :])
                w2_sb.append(t)
            w3_sb = cpool.tile([128, 64], F32R, tag="w3")
            nc.sync.dma_start(w3_sb[:], w3_d[:, :])
            b1_sb = []
            for m in range(2):
                t = cpool.tile([128, 1], F32, tag=f"b1{m}", name=f"b1s{m}")
                nc.sync.dma_start(t[:], b1_d[128 * m : 128 * (m + 1), :])
                b1_sb.append(t)
            b2_sb = cpool.tile([128, 1], F32, tag="b2")
            nc.sync.dma_start(b2_sb[:], b2_d[:, :])
            b3_sb = cpool.tile([64, 1], F32, tag="b3")
            nc.sync.dma_start(b3_sb[:], b3_d[:, :])

            bags_sb = cpool.tile(bags_shape, F16, tag="bags")
            nc.sync.dma_start(bags_sb[:], bags_d[:, :])

            g1_sb = [
                cpool.tile([128, N], F32R, tag=f"g1{m}", name=f"g1s{m}")
                for m in range(2)
            ]
            g2_sb = cpool.tile([128, N], F32R, tag="g2")
            hT_sb = cpool.tile([128, N], F16, tag="hT")
            colmax_sb = cpool.tile([128, NT], F32, tag="colmax")

            # ---- layer 1: g1 = relu(W1.T @ xT + b1) -> [256, 2048] as 2 tiles
            for m in range(2):
                ps = ppool.tile([128, N], F32, tag="ps", name=f"psl1{m}")
                for j in range(4):
                    for k in range(4):
                        _mm(
                            nc,
                            ps[:, 512 * j : 512 * (j + 1)],
                            w1_sb[k][:, 128 * m : 128 * (m + 1)],
                            xT_sb[k][:, 512 * j : 512 * (j + 1)],
                            start=(k == 0),
                            stop=(k == 3),
                        )
                nc.scalar.activation(g1_sb[m][:, :], ps[:, :], relu, bias=b1_sb[m][:, :])

            # ---- layer 2: g2 = relu(W2.T @ g1 + b2) -> [128, 2048]
            ps = ppool.tile([128, N], F32, tag="ps", name="psl2")
            for j in range(4):
                for k in range(2):
                    _mm(
                        nc,
                        ps[:, 512 * j : 512 * (j + 1)],
                        w2_sb[k][:, :],
                        g1_sb[k][:, 512 * j : 512 * (j + 1)],
                        start=(k == 0),
                        stop=(k == 1),
                    )
            nc.scalar.activation(g2_sb[:, :], ps[:, :], relu, bias=b2_sb[:, :])

            # ---- layer 3: hT = relu(W3.T @ g2 + b3) -> [64, 2048], then
            #      duplicated into partitions 64:128 for row-group alternation
            ps = ppool.tile([128, N], F32, tag="ps", name="psl3")
            for j in range(4):
                _mm(
                    nc,
                    ps[0:64, 512 * j : 512 * (j + 1)],
                    w3_sb[:, :],
                    g2_sb[:, 512 * j : 512 * (j + 1)],
                    start=True,
                    stop=True,
                )
            nc.scalar.activation(hT_sb[0:64, :], ps[0:64, :], relu, bias=b3_sb[:, :])
            if USE_ALT:
                nc.sync.dma_start(hT_sb[64:128, :], hT_sb[0:64, :])

            # ---- scores: tile i lives in partition half i%2, col block i//2
            for i in range(NT):
                if USE_ALT:
                    half = 64 * (i % 2)
                    lhsT = bags_sb[half : half + 64, 128 * (i // 2) : 128 * (i // 2) + 128]
                else:
                    half = 0
                    lhsT = bags_sb[:, 128 * i : 128 * (i + 1)]
                ps = ppool.tile([128, N], F32, tag="ps", name=f"pss{i}")
                for j in range(4):
                    _mm(
                        nc,
                        ps[:, 512 * j : 512 * (j + 1)],
                        lhsT,
                        hT_sb[half : half + 64, 512 * j : 512 * (j + 1)],
                        start=True,
                        stop=True,
                    )
                if USE_SPLIT and (i % 10) < 8:
                    scr = spool.tile([128, N], F16, tag="scr", name=f"scr{i}")
                    nc.scalar.activation(scr[:, :], ps[:, :], copyf)
                    t1 = spool.tile([128, N // 2], F16, tag="t1", name=f"t1_{i}")
                    nc.vector.tensor_max(t1[:, :], scr[:, 0 : N // 2], scr[:, N // 2 : N])
                    t2 = spool.tile([128, N // 4], F16, tag="t2", name=f"t2_{i}")
                    nc.vector.tensor_max(t2[:, :], t1[:, 0 : N // 4], t1[:, N // 4 : N // 2])
                    nc.vector.reduce_max(
                        colmax_sb[:, i : i + 1], t2[:, :], axis=mybir.AxisListType.X
                    )
                else:
                    nc.vector.reduce_max(
                        colmax_sb[:, i : i + 1], ps[:, :], axis=mybir.AxisListType.X
                    )

            nc.sync.dma_start(out_d[:, :], colmax_sb[:])

    nc.compile()
    return nc


_CACHED = {}


def _get_program():
    if "nc" not in _CACHED:
        _CACHED["nc"] = _build_program()
    return _CACHED["nc"]


def _run_device(in_maps, trace=False):
    nc = _get_program()
    try:
        return run_bass_kernel_spmd(nc, in_maps, list(range(NCORES)), trace=trace)
    except ModuleNotFoundError:
        if not trace:
            raise
        return run_bass_kernel_spmd(nc, in_maps, list(range(NCORES)), trace=False)


def _prep_inputs(x, tr_bags, W1, b1, W2, b2, W3, b3):
    xT = np.ascontiguousarray(np.asarray(x, np.float32)[0].T)  # [512, 2048]
    bags = np.asarray(tr_bags, np.float32)
    bags_pad = np.zeros((64, NCORES * TPC), np.float32)
    bags_pad[:, :T] = bags
    base = {
        "xT": xT,
        "w1": np.ascontiguousarray(np.asarray(W1, np.float32)),
        "w2": np.ascontiguousarray(np.asarray(W2, np.float32)),
        "w3": np.ascontiguousarray(np.asarray(W3, np.float32)),
        "b1": np.asarray(b1, np.float32).reshape(256, 1).copy(),
        "b2": np.asarray(b2, np.float32).reshape(128, 1).copy(),
        "b3": np.asarray(b3, np.float32).reshape(64, 1).copy(),
    }
    in_maps = []
    for c in range(NCORES):
        shard = bags_pad[:, c * TPC : (c + 1) * TPC]
        if USE_ALT:
            sh = shard.reshape(64, NT, 128)
            packed = np.empty((128, TPC // 2), np.float32)
            # even tiles -> partitions 0:64, odd tiles -> 64:128, col block i//2
            packed[0:64] = sh[:, 0::2, :].reshape(64, -1)
            packed[64:128] = sh[:, 1::2, :].reshape(64, -1)
        else:
            packed = shard
        m = dict(base)
        m["bags"] = np.ascontiguousarray(packed.astype(np.float16))
        in_maps.append(m)
    return in_maps


def _finish_host(colmax, tr_mask, W4, b4):
    tm = np.asarray(tr_mask)
    boundaries = np.searchsorted(tm, np.arange(R + 1))
    ref_max = np.full(R, -np.inf, np.float32)
    nonempty = boundaries[1:] > boundaries[:-1]
    if nonempty.any():
        starts = boundaries[:-1][nonempty]
        ref_max[nonempty] = np.maximum.reduceat(colmax, starts)[: nonempty.sum()]
    z = ref_max.astype(np.float32) @ np.asarray(W4, np.float32) + np.asarray(
        b4, np.float32
    )
    y_prob = (1.0 / (1.0 + np.exp(-z.astype(np.float64)))).astype(np.float32).squeeze()
    y_hat = np.float32(1.0) if y_prob >= 0.5 else np.float32(0.0)
    return np.asarray(y_prob, np.float32), np.asarray(y_hat, np.float32)


def kernel(x, tr_bags, tr_mask, W1, b1, W2, b2, W3, b3, W4, b4, _trace=False):
    in_maps = _prep_inputs(x, tr_bags, W1, b1, W2, b2, W3, b3)
    res = _run_device(in_maps, trace=_trace)
    colmax_parts = []
    for c in range(NCORES):
        cm = res.results[c]["colmax_out"]  # [128, NT]
        colmax_parts.append(np.asarray(cm).T.reshape(-1))  # [TPC], col-major by tile
    colmax = np.concatenate(colmax_parts)[:T]
    out = _finish_host(colmax, tr_mask, W4, b4)
    if _trace:
        return out, res
    return out


# revision 11
# speedup vs baseline: 1.7188x; 1.1015x over previous
"""Trainium2 Bass kernel for nn_BSN_76218489635087 (segment_reduce).

Computation (reference):
    h = relu-MLP(x[0])            # [2048, 64]
    s = h @ tr_bags               # [2048, 100000]
    col_max = max over rows       # [100000]
    ref_max = segment_max(col_max, tr_mask, 100)
    y_prob = sigmoid(ref_max @ W4 + b4); y_hat = y_prob >= 0.5

Sharding: tr_bags columns (T) split across 8 cores (12544 padded cols each).
Each core computes the full (replicated) MLP producing hT = h.T, duplicated
into both partition halves [128, 2048]. Bags are host-packed [128, 6272] so
consecutive 128-col tiles alternate partition halves (row groups), letting
LDWEIGHTS overlap in-flight MATMULs. Matmuls run in float32r (full fp32
operands, 1 cycle/row for N>=256 vs 4 for plain fp32).

PSUM drain (the bottleneck): per score tile [128, 2048] either
  - DVE reduce_max direct from PSUM, or
  - ACT copies PSUM -> SBUF fp16, then one DVE tensor_tensor_reduce(max,max)
    folds the 2048 fp16 values to the per-column max,
split ~5:3 so ScalarE and VectorE drain concurrently.

Host gathers the 100352 column maxes, does the segment-max + final
100->1 dot + sigmoid.
"""

import sys
import os

for _p in ("/opt/trn_rl_repo", "/root/.axon_site/_ro/pypackages", "/root/.axon_site"):
    if _p not in sys.path and os.path.isdir(_p):
        sys.path.append(_p)

import numpy as np

from concourse import bass, bacc, tile, mybir
from concourse.bass_utils import run_bass_kernel_spmd

# Problem constants (hardcoded per harness contract)
N = 2048          # instances
D = 512           # input features
T = 100000        # reference instance columns
R = 100           # num references (segments)
NCORES = 8
TPC = 12544       # padded columns per core (= 98 * 128); 8*12544 = 100352
NT = TPC // 128   # 98 column-tiles per core

F32 = mybir.dt.float32
F32R = mybir.dt.float32r
F16 = mybir.dt.float16

# Of every 8 score tiles, this many drain via the ACT-copy path (rest DVE).
ACT_TILES = frozenset({0, 1, 2, 4, 6})

USE_ALT = os.environ.get("K_ALT", "0") == "1"      # row-group alternation
USE_SPLIT = os.environ.get("K_SPLIT", "1") == "1"  # ACT/DVE drain split
TAIL = os.environ.get("K_TAIL", "ttr")             # ttr | reduce


def _mm(nc, out, lhsT, rhs, **kw):
    nc.tensor.matmul(out, lhsT, rhs, **kw)


def _build_program():
    nc = bacc.Bacc("TRN2", target_bir_lowering=False, debug=False, num_devices=NCORES)

    xT_d = nc.dram_tensor("xT", [D, N], F16, kind="ExternalInput")
    w1_d = nc.dram_tensor("w1", [D, 256], F16, kind="ExternalInput")
    w2_d = nc.dram_tensor("w2", [256, 128], F16, kind="ExternalInput")
    w3_d = nc.dram_tensor("w3", [128, 64], F16, kind="ExternalInput")
    b1_d = nc.dram_tensor("b1", [256, 1], F32, kind="ExternalInput")
    b2_d = nc.dram_tensor("b2", [128, 1], F32, kind="ExternalInput")
    b3_d = nc.dram_tensor("b3", [64, 1], F32, kind="ExternalInput")
    bags_shape = [128, TPC // 2] if USE_ALT else [64, TPC]
    bags_d = nc.dram_tensor("bags", bags_shape, F16, kind="ExternalInput")
    out_d = nc.dram_tensor("colmax_out", [128, NT], F32, kind="ExternalOutput")

    relu = mybir.ActivationFunctionType.Relu
    copyf = mybir.ActivationFunctionType.Copy
    amax = mybir.AluOpType.max

    with tile.TileContext(nc) as tc:
        with (
            tc.tile_pool(name="const", bufs=1) as cpool,
            tc.tile_pool(name="scr", bufs=4) as spool,
            tc.tile_pool(name="psum", bufs=2, space="PSUM") as ppool,
        ):
            # ---- load everything ----
            xT_sb = []
            for k in range(4):
                t = cpool.tile([128, N], F16, tag=f"xT{k}", name=f"xT{k}")
                nc.sync.dma_start(t[:], xT_d[128 * k : 128 * (k + 1), :])
                xT_sb.append(t)
            w1_sb = []
            for k in range(4):
                t = cpool.tile([128, 256], F16, tag=f"w1{k}", name=f"w1s{k}")
                nc.sync.dma_start(t[:], w1_d[128 * k : 128 * (k + 1), :])
                w1_sb.append(t)
            w2_sb = []
            for k in range(2):
                t = cpool.tile([128, 128], F16, tag=f"w2{k}", name=f"w2s{k}")
                nc.sync.dma_start(t[:], w2_d[128 * k : 128 * (k + 1), :])
                w2_sb.append(t)
            w3_sb = cpool.tile([128, 64], F16, tag="w3")
            nc.sync.dma_start(w3_sb[:], w3_d[:, :])
            b1_sb = []
            for m in range(2):
                t = cpool.tile([128, 1], F32, tag=f"b1{m}", name=f"b1s{m}")
                nc.sync.dma_start(t[:], b1_d[128 * m : 128 * (m + 1), :])
                b1_sb.append(t)
            b2_sb = cpool.tile([128, 1], F32, tag="b2")
            nc.sync.dma_start(b2_sb[:], b2_d[:, :])
            b3_sb = cpool.tile([64, 1], F32, tag="b3")
            nc.sync.dma_start(b3_sb[:], b3_d[:, :])

            bags_sb = cpool.tile(bags_shape, F16, tag="bags")
            nc.sync.dma_start(bags_sb[:], bags_d[:, :])

            g1_sb = [
                cpool.tile([128, N], F16, tag=f"g1{m}", name=f"g1s{m}")
                for m in range(2)
            ]
            g2_sb = cpool.tile([128, N], F16, tag="g2")
            hT_sb = cpool.tile([128, N], F16, tag="hT")
            colmax_sb = cpool.tile([128, NT], F32, tag="colmax")

            # ---- layer 1: g1 = relu(W1.T @ xT + b1) -> [256, 2048] as 2 tiles
            for m in range(2):
                ps = ppool.tile([128, N], F32, tag="ps", name=f"psl1{m}")
                for j in range(4):
                    for k in range(4):
                        _mm(
                            nc,
                            ps[:, 512 * j : 512 * (j + 1)],
                            w1_sb[k][:, 128 * m : 128 * (m + 1)],
                            xT_sb[k][:, 512 * j : 512 * (j + 1)],
                            start=(k == 0),
                            stop=(k == 3),
                        )
                nc.scalar.activation(g1_sb[m][:, :], ps[:, :], relu, bias=b1_sb[m][:, :])

            # ---- layer 2: g2 = relu(W2.T @ g1 + b2) -> [128, 2048]
            ps = ppool.tile([128, N], F32, tag="ps", name="psl2")
            for j in range(4):
                for k in range(2):
                    _mm(
                        nc,
                        ps[:, 512 * j : 512 * (j + 1)],
                        w2_sb[k][:, :],
                        g1_sb[k][:, 512 * j : 512 * (j + 1)],
                        start=(k == 0),
                        stop=(k == 1),
                    )
            nc.scalar.activation(g2_sb[:, :], ps[:, :], relu, bias=b2_sb[:, :])

            # ---- layer 3: hT = relu(W3.T @ g2 + b3) -> [64, 2048], then
            #      duplicated into partitions 64:128 for row-group alternation
            ps = ppool.tile([128, N], F32, tag="ps", name="psl3")
            for j in range(4):
                _mm(
                    nc,
                    ps[0:64, 512 * j : 512 * (j + 1)],
                    w3_sb[:, :],
                    g2_sb[:, 512 * j : 512 * (j + 1)],
                    start=True,
                    stop=True,
                )
            nc.scalar.activation(hT_sb[0:64, :], ps[0:64, :], relu, bias=b3_sb[:, :])
            if USE_ALT:
                nc.sync.dma_start(hT_sb[64:128, :], hT_sb[0:64, :])

            # ---- scores: tile i lives in partition half i%2, col block i//2
            for i in range(NT):
                if USE_ALT:
                    half = 64 * (i % 2)
                    lhsT = bags_sb[half : half + 64, 128 * (i // 2) : 128 * (i // 2) + 128]
                else:
                    half = 0
                    lhsT = bags_sb[:, 128 * i : 128 * (i + 1)]
                ps = ppool.tile([128, N], F32, tag="ps", name=f"pss{i}")
                for j in range(4):
                    _mm(
                        nc,
                        ps[:, 512 * j : 512 * (j + 1)],
                        lhsT,
                        hT_sb[half : half + 64, 512 * j : 512 * (j + 1)],
                        start=True,
                        stop=True,
                    )
                if USE_SPLIT and (i % 7) < 6:
                    scr = spool.tile([128, N], F16, tag="scr", name=f"scr{i}")
                    nc.scalar.activation(scr[:, :], ps[:, :], copyf)
                    t1 = spool.tile([128, N // 2], F16, tag="t1", name=f"t1_{i}")
                    nc.vector.tensor_max(t1[:, :], scr[:, 0 : N // 2], scr[:, N // 2 : N])
                    t2 = spool.tile([128, N // 4], F16, tag="t2", name=f"t2_{i}")
                    nc.vector.tensor_max(t2[:, :], t1[:, 0 : N // 4], t1[:, N // 4 : N // 2])
                    nc.vector.reduce_max(
                        colmax_sb[:, i : i + 1], t2[:, :], axis=mybir.AxisListType.X
                    )
                else:
                    nc.vector.reduce_max(
                        colmax_sb[:, i : i + 1], ps[:, :], axis=mybir.AxisListType.X
                    )

            nc.sync.dma_start(out_d[:, :], colmax_sb[:])

    nc.compile()
    return nc


_CACHED = {}


def _get_program():
    if "nc" not in _CACHED:
        _CACHED["nc"] = _build_program()
    return _CACHED["nc"]


def _run_device(in_maps, trace=False):
    nc = _get_program()
    try:
        return run_bass_kernel_spmd(nc, in_maps, list(range(NCORES)), trace=trace)
    except ModuleNotFoundError:
        if not trace:
            raise
        return run_bass_kernel_spmd(nc, in_maps, list(range(NCORES)), trace=False)


def _prep_inputs(x, tr_bags, W1, b1, W2, b2, W3, b3):
    xT = np.ascontiguousarray(np.asarray(x, np.float32)[0].T)  # [512, 2048]
    bags = np.asarray(tr_bags, np.float32)
    bags_pad = np.zeros((64, NCORES * TPC), np.float32)
    bags_pad[:, :T] = bags
    base = {
        "xT": xT.astype(np.float16),
        "w1": np.ascontiguousarray(np.asarray(W1, np.float32).astype(np.float16)),
        "w2": np.ascontiguousarray(np.asarray(W2, np.float32).astype(np.float16)),
        "w3": np.ascontiguousarray(np.asarray(W3, np.float32).astype(np.float16)),
        "b1": np.asarray(b1, np.float32).reshape(256, 1).copy(),
        "b2": np.asarray(b2, np.float32).reshape(128, 1).copy(),
        "b3": np.asarray(b3, np.float32).reshape(64, 1).copy(),
    }
    in_maps = []
    for c in range(NCORES):
        shard = bags_pad[:, c * TPC : (c + 1) * TPC]
        if USE_ALT:
            sh = shard.reshape(64, NT, 128)
            packed = np.empty((128, TPC // 2), np.float32)
            # even tiles -> partitions 0:64, odd tiles -> 64:128, col block i//2
            packed[0:64] = sh[:, 0::2, :].reshape(64, -1)
            packed[64:128] = sh[:, 1::2, :].reshape(64, -1)
        else:
            packed = shard
        m = dict(base)
        m["bags"] = np.ascontiguousarray(packed.astype(np.float16))
        in_maps.append(m)
    return in_maps


def _finish_host(colmax, tr_mask, W4, b4):
    tm = np.asarray(tr_mask)
    boundaries = np.searchsorted(tm, np.arange(R + 1))
    ref_max = np.full(R, -np.inf, np.float32)
    nonempty = boundaries[1:] > boundaries[:-1]
    if nonempty.any():
        starts = boundaries[:-1][nonempty]
        ref_max[nonempty] = np.maximum.reduceat(colmax, starts)[: nonempty.sum()]
    z = ref_max.astype(np.float32) @ np.asarray(W4, np.float32) + np.asarray(
        b4, np.float32
    )
    y_prob = (1.0 / (1.0 + np.exp(-z.astype(np.float64)))).astype(np.float32).squeeze()
    y_hat = np.float32(1.0) if y_prob >= 0.5 else np.float32(0.0)
    return np.asarray(y_prob, np.float32), np.asarray(y_hat, np.float32)


def kernel(x, tr_bags, tr_mask, W1, b1, W2, b2, W3, b3, W4, b4, _trace=False):
    in_maps = _prep_inputs(x, tr_bags, W1, b1, W2, b2, W3, b3)
    res = _run_device(in_maps, trace=_trace)
    colmax_parts = []
    for c in range(NCORES):
        cm = res.results[c]["colmax_out"]  # [128, NT]
        colmax_parts.append(np.asarray(cm).T.reshape(-1))  # [TPC], col-major by tile
    colmax = np.concatenate(colmax_parts)[:T]
    out = _finish_host(colmax, tr_mask, W4, b4)
    if _trace:
        return out, res
    return out
